# revision 1
# baseline (speedup 1.0000x reference)
"""Multi-head attention (B=2, S=4096, D=512, H=8) on 8 TRN2 NeuronCores.

Sharding: core = (batch, query-chunk-of-1024). Each core recomputes the K/V
projections for its batch (no collectives), runs flash-style attention over
its query chunk, and the output projection. Host splits/gathers.
"""
import os
import sys
import time

for _p in ("/opt/trn_rl_repo",):
    if _p not in sys.path:
        sys.path.insert(0, _p)

import numpy as np
from contextlib import ExitStack

import concourse.bass as bass
import concourse.bacc as bacc
import concourse.tile as tile
from concourse import mybir
from concourse.bass_utils import run_bass_kernel_spmd

F16 = mybir.dt.float16
F32 = mybir.dt.float32

D = 512          # d_model
H = 8            # heads
DK = 64          # head dim
SQ = 1024        # queries per core
SK = 4096        # keys (full batch sequence)
NCORES = 8
NQT = SQ // 512          # 512-wide query tiles per core (2)
NCHUNK = SK // 128       # 128-key chunks (32)
NGROUP = SK // 512       # kv-projection groups of 512 keys (8)

LAST_RESULTS = None      # BassKernelResults of the most recent run (for test.py)


def _build_kernel():
    nc = bacc.Bacc("TRN2", target_bir_lowering=False, debug=False,
                   num_devices=NCORES)

    qT = nc.dram_tensor("qT", [D, SQ], F16, kind="ExternalInput").ap()
    kT = nc.dram_tensor("kT", [D, SK], F16, kind="ExternalInput").ap()
    vT = nc.dram_tensor("vT", [D, SK], F16, kind="ExternalInput").ap()
    wq = nc.dram_tensor("wq", [D, D], F16, kind="ExternalInput").ap()
    wk = nc.dram_tensor("wk", [D, D], F16, kind="ExternalInput").ap()
    wv = nc.dram_tensor("wv", [D, D], F16, kind="ExternalInput").ap()
    wo = nc.dram_tensor("wo", [D, D], F16, kind="ExternalInput").ap()
    bq = nc.dram_tensor("bq", [D], F32, kind="ExternalInput").ap()
    bk = nc.dram_tensor("bk", [D], F32, kind="ExternalInput").ap()
    bv = nc.dram_tensor("bv", [D], F32, kind="ExternalInput").ap()
    bo = nc.dram_tensor("bo", [D], F32, kind="ExternalInput").ap()
    out = nc.dram_tensor("out", [SQ, D], F32, kind="ExternalOutput").ap()

    with tile.TileContext(nc) as tc:
        _emit(tc, qT, kT, vT, wq, wk, wv, wo, bq, bk, bv, bo, out)

    nc.compile()
    return nc


def _emit(tc, qT, kT, vT, wq, wk, wv, wo, bq, bk, bv, bo, out):
    nc = tc.nc
    Exp = mybir.ActivationFunctionType.Exp

    with ExitStack() as ctx:
        const = ctx.enter_context(tc.tile_pool(name="const", bufs=1))
        kvin = ctx.enter_context(tc.tile_pool(name="kvin", bufs=3))
        ptp = ctx.enter_context(tc.tile_pool(name="ptp", bufs=5))
        normp = ctx.enter_context(tc.tile_pool(name="normp", bufs=2))
        outp = ctx.enter_context(tc.tile_pool(name="outp", bufs=2))
        # 3 x [128,1024] st slots (6 banks) + 2 pvp banks = 8 PSUM banks.
        # All other PSUM users (projections, oproj) borrow "st" slots.
        stp = ctx.enter_context(tc.tile_pool(name="stp", bufs=3, space="PSUM"))
        pvpp = ctx.enter_context(tc.tile_pool(name="pvpp", bufs=2, space="PSUM"))
        pjp = stp

        # ---- persistent SBUF tensors -------------------------------------
        wq_sb = const.tile([128, 4 * D], F16)     # [dm%128, m*512 + d]
        wk_sb = const.tile([128, 4 * D], F16)
        wv_sb = const.tile([128, 4 * D], F16)
        wo_sb = const.tile([128, 4 * D], F16)     # [d%128, t*512 + e]
        qin = const.tile([128, 4 * SQ], F16)      # [dm%128, m*1024 + qq]
        qhT = const.tile([128, 4 * SQ], F16)      # [d%128, t*1024 + qq]
        khT = const.tile([128, 4 * SK], F16)      # [d%128, t*4096 + kk]
        vh = const.tile([128, NCHUNK * (H * 65)], F16)  # per chunk: h*65+dd, col 64=1
        cat = const.tile([128, 4 * SQ], F16)      # [d%128, t*1024 + qq]
        bqk = const.tile([128, 8], F32)           # cols 0-3 bq tiles, 4-7 bk
        bvo = const.tile([1, 2 * D], F32)         # bv | bo
        bv_rep = const.tile([128, D], F32)
        bo_rep = const.tile([128, D], F32)
        ones32 = const.tile([1, 128], F32)

        # ---- startup loads: spread issue across idle engine DMA queues ---
        for m in range(4):
            nc.sync.dma_start(wk_sb[:, m * D:(m + 1) * D], wk[m * 128:(m + 1) * 128, :])
            nc.scalar.dma_start(wq_sb[:, m * D:(m + 1) * D], wq[m * 128:(m + 1) * 128, :])
            nc.scalar.dma_start(qin[:, m * SQ:m * SQ + 512], qT[m * 128:(m + 1) * 128, 0:512])
        nc.sync.dma_start(bqk[:, 0:4], bq.rearrange("(t p) -> p t", p=128))
        nc.sync.dma_start(bqk[:, 4:8], bk.rearrange("(t p) -> p t", p=128))
        for m in range(4):
            nc.scalar.dma_start(wv_sb[:, m * D:(m + 1) * D], wv[m * 128:(m + 1) * 128, :])
        nc.scalar.dma_start(bvo[0:1, 0:D], bv.rearrange("(a d) -> a d", a=1))
        nc.scalar.dma_start(bvo[0:1, D:2 * D], bo.rearrange("(a d) -> a d", a=1))
        for m in range(4):
            nc.scalar.dma_start(qin[:, m * SQ + 512:(m + 1) * SQ],
                                qT[m * 128:(m + 1) * 128, 512:SQ])
        for m in range(4):
            nc.scalar.dma_start(wo_sb[:, m * D:(m + 1) * D], wo[m * 128:(m + 1) * 128, :])
        nc.vector.memset(ones32[:], 1.0)
        # ones column of every vh chunk (col 64 of each head block)
        vh_ones = vh[:, :].rearrange("p (a c) -> p a c", c=65)[:, :, 64:65]
        nc.vector.memset(vh_ones, 1.0)
        # preload the exp table set while startup DMAs run
        warm = normp.tile([1, 128], F16, tag="warm")
        nc.scalar.activation(warm[0:1, :], ones32[0:1, :], Exp)

        # replicate bv / bo across partitions (gpsimd broadcast)
        for src_off, rep in ((0, bv_rep), (D, bo_rep)):
            nc.gpsimd.partition_broadcast(rep[:, :], bvo[0:1, src_off:src_off + D])

        # ---- Q projection group: qhT[d, q] = Wq @ qT + bq ----------------
        def emit_qproj(t, qt):
            pj = pjp.tile([128, D], F32, tag="st")
            for m in range(4):
                nc.tensor.matmul(
                    pj[:],
                    wq_sb[:, m * D + t * 128: m * D + (t + 1) * 128],
                    qin[:, m * SQ + qt * 512: m * SQ + qt * 512 + 512],
                    start=(m == 0), stop=(m == 3))
            nc.vector.tensor_scalar_add(
                qhT[:, t * SQ + qt * 512: t * SQ + qt * 512 + 512],
                pj[:], bqk[:, t:t + 1])

        # ---- K/V projection: DMA prefetch + 8 fine-grained MM steps ------
        kv_tiles = {}

        def prefetch_kv(G, eng=None):
            keng = eng or nc.sync
            veng = eng or nc.sync
            kin = kvin.tile([128, 4 * 512], F16, tag="kin", name="kin")
            vin = kvin.tile([128, 4 * 512], F16, tag="vin", name="vin")
            kv_tiles[G] = (kin, vin)
            for m in range(4):
                keng.dma_start(kin[:, m * 512:(m + 1) * 512],
                               kT[m * 128:(m + 1) * 128, G * 512:(G + 1) * 512])
            for m in range(4):
                veng.dma_start(vin[:, m * 512:(m + 1) * 512],
                               vT[m * 128:(m + 1) * 128, G * 512:(G + 1) * 512])

        def kvproj_step(G, k):
            kin, vin = kv_tiles[G]
            if k < 4:
                # khT[d, kk] = Wk @ kT + bk  (d-tile t = k)
                t = k
                pj = pjp.tile([128, 512], F32, tag="st", name="pj")
                for m in range(4):
                    nc.tensor.matmul(
                        pj[:],
                        wk_sb[:, m * D + t * 128: m * D + (t + 1) * 128],
                        kin[:, m * 512:(m + 1) * 512],
                        start=(m == 0), stop=(m == 3))
                nc.vector.tensor_scalar_add(
                    khT[:, t * SK + G * 512: t * SK + G * 512 + 512],
                    pj[:], bqk[:, 4 + t:5 + t])
            else:
                # vh[kk, h*65+dd] = (vT.T @ WvT + bv), strided, ones col kept
                ci = k - 4
                cc = 4 * G + ci
                pj = pjp.tile([128, 512], F32, tag="st", name="pj")
                for m in range(4):
                    nc.tensor.matmul(
                        pj[:],
                        vin[:, m * 512 + ci * 128: m * 512 + ci * 128 + 128],
                        wv_sb[:, m * D:(m + 1) * D],
                        start=(m == 0), stop=(m == 3))
                dst = vh[:, cc * (H * 65):(cc + 1) * (H * 65)]
                dst = dst.rearrange("p (h c) -> p h c", c=65)[:, :, 0:64]
                nc.vector.tensor_add(
                    dst,
                    pj[:].rearrange("p (h c) -> p h c", c=64),
                    bv_rep[:].rearrange("p (h c) -> p h c", c=64))


        # ---- output projection for one 128-query block -------------------
        def emit_oproj(qt, i):
            qb = qt * 512 + i * 128
            pj = pjp.tile([128, 512], F32, tag="st")
            for t in range(4):
                nc.tensor.matmul(
                    pj[:],
                    cat[:, t * SQ + qb: t * SQ + qb + 128],
                    wo_sb[:, t * D:(t + 1) * D],
                    start=(t == 0), stop=(t == 3))
            ob = outp.tile([128, 512], F32, tag="ob")
            nc.vector.tensor_add(ob[:], pj[:], bo_rep[:])
            eng = (nc.sync, nc.gpsimd, nc.scalar)[i % 3]
            eng.dma_start(out[qb:qb + 128, :], ob[:])

        # ---- attention: software-pipelined over 256 head-granules --------
        # unit u = (sweep s = u//32, j = u%2, g = (u%32)//2); sweep s =
        # (qt, hp) = divmod(s, 4) ... qt-major: s = qt*4 + hp.
        # Stages: SC at u, exp at u-2, PV at u-4 (PV first in each step).
        NU = 8 * 32
        pvp_tiles = {}
        st_tiles = {}
        pt_tiles = {}

        def unit(u):
            s, r = divmod(u, 32)
            qt, hp = divmod(s, 4)
            return qt, hp, r // 2, r % 2   # qt, head-pair, granule, head-idx

        def emit_sc(u):
            qt, hp, g, j = unit(u)
            qs = qt * 512
            stt = stp.tile([128, 1024], F32, tag="st")
            st_tiles[u] = stt
            lo, hi = 64 * j, 64 * j + 64
            for ci in range(2):
                cc = 2 * g + ci
                nc.tensor.matmul(
                    stt[:, ci * 512:(ci + 1) * 512],
                    khT[lo:hi, hp * SK + cc * 128: hp * SK + (cc + 1) * 128],
                    qhT[lo:hi, hp * SQ + qs: hp * SQ + qs + 512],
                    tile_position=(64 * j, 0))

        def emit_act(u):
            ptt = ptp.tile([128, 1024], F16, tag="pt")
            pt_tiles[u] = ptt
            nc.scalar.activation(ptt[:], st_tiles.pop(u)[:], Exp, scale=0.125)

        def emit_pv(u):
            qt, hp, g, j = unit(u)
            h = 2 * hp + j
            if g == 0:
                pvp_tiles[u % 2] = pvpp.tile([128, 512], F32, tag="pvp", name="pvp")
            pvp = pvp_tiles[u % 2]
            ptt = pt_tiles.pop(u)
            for ci in range(2):
                cc = 2 * g + ci
                nc.tensor.matmul(
                    pvp[0:65, :],
                    vh[:, cc * (H * 65) + h * 65: cc * (H * 65) + h * 65 + 65],
                    ptt[:, ci * 512:(ci + 1) * 512],
                    start=(g == 0 and ci == 0), stop=(g == 15 and ci == 1))
            if g == 15:
                # normalize: cat[dd, h*1024+q] = pvp[0:64] / pvp[64]
                qs = qt * 512
                sums = normp.tile([1, 512], F32, tag="sums")
                nc.vector.tensor_copy(sums[0:1, :], pvp[64:65, :])
                rec = normp.tile([1, 512], F32, tag="rec")
                nc.vector.reciprocal_approx_fast(rec[0:1, :], sums[0:1, :])
                rep_sb = normp.tile([64, 512], F32, tag="rep")
                nc.gpsimd.partition_broadcast(rep_sb[:, :], rec[0:1, :])
                lo = (h % 2) * 64
                nc.vector.tensor_mul(
                    cat[lo:lo + 64, (h // 2) * SQ + qs: (h // 2) * SQ + qs + 512],
                    pvp[0:64, :], rep_sb[:])

        prefetch_kv(0, eng=nc.gpsimd)
        prefetch_kv(1, eng=nc.sync)
        prefetch_kv(2, eng=nc.gpsimd)

        for u in range(NU + 4):
            if u < NU:
                qt, hp, g, j = unit(u)
                if g == 0 and j == 0:
                    emit_qproj(hp, qt)          # qhT slice for this sweep
                if u == 0:
                    for k in range(8):
                        kvproj_step(0, k)       # group 0 fully up front
                if u % 4 == 1 and (u - 1) // 4 + 3 <= NGROUP - 1:
                    prefetch_kv((u - 1) // 4 + 3)   # DMA two groups ahead
                if 1 <= u <= 4 * (NGROUP - 1):
                    G = (u - 1) // 4 + 1        # due before unit 4G (hook-first)
                    for k in range(2 * ((u - 1) % 4), 2 * ((u - 1) % 4) + 2):
                        kvproj_step(G, k)
                if u % 32 == 8 and (u // 32) in (4, 5, 6, 7):
                    # spread qt0's output projection into sweeps 4..7
                    emit_oproj(0, u // 32 - 4)
            if u >= 4:
                emit_pv(u - 4)
            if u < NU:
                emit_sc(u)
            if u >= 2 and u - 2 < NU:
                emit_act(u - 2)

        for i in range(4):
            emit_oproj(1, i)


_NC_CACHE = None


def _get_nc():
    global _NC_CACHE
    if _NC_CACHE is None:
        _NC_CACHE = _build_kernel()
    return _NC_CACHE


def kernel(q, k, v, Wq, bq, Wk, bk, Wv, bv, Wo, bo, trace=False):
    global LAST_RESULTS
    q = np.asarray(q, np.float32)
    k = np.asarray(k, np.float32)
    v = np.asarray(v, np.float32)

    kT16 = [np.ascontiguousarray(k[b].T).astype(np.float16) for b in range(2)]
    vT16 = [np.ascontiguousarray(v[b].T).astype(np.float16) for b in range(2)]
    wq16 = np.ascontiguousarray(np.asarray(Wq, np.float32).T).astype(np.float16)
    wk16 = np.ascontiguousarray(np.asarray(Wk, np.float32).T).astype(np.float16)
    wv16 = np.ascontiguousarray(np.asarray(Wv, np.float32).T).astype(np.float16)
    wo16 = np.ascontiguousarray(np.asarray(Wo, np.float32).T).astype(np.float16)
    bq32 = np.ascontiguousarray(np.asarray(bq, np.float32))
    bk32 = np.ascontiguousarray(np.asarray(bk, np.float32))
    bv32 = np.ascontiguousarray(np.asarray(bv, np.float32))
    bo32 = np.ascontiguousarray(np.asarray(bo, np.float32))

    in_maps = []
    for core in range(NCORES):
        b, c = divmod(core, 4)
        qT16 = np.ascontiguousarray(
            q[b, c * SQ:(c + 1) * SQ, :].T).astype(np.float16)
        in_maps.append({
            "qT": qT16, "kT": kT16[b], "vT": vT16[b],
            "wq": wq16, "wk": wk16, "wv": wv16, "wo": wo16,
            "bq": bq32, "bk": bk32, "bv": bv32, "bo": bo32,
        })

    nc = _get_nc()
    res = run_bass_kernel_spmd(nc, in_maps, core_ids=list(range(NCORES)),
                               trace=trace)
    LAST_RESULTS = res

    full = np.empty((2, SK, D), np.float32)
    for core in range(NCORES):
        b, c = divmod(core, 4)
        full[b, c * SQ:(c + 1) * SQ, :] = res.results[core]["out"]
    return full



# revision 4
# speedup vs baseline: 1.0314x; 1.0314x over previous
"""Multi-head attention (B=2, S=4096, D=512, H=8) on 8 TRN2 NeuronCores.

Sharding: core = (batch, head-pair). Each core projects q/k/v onto its two
heads' 128 dims over the full 4096-token sequence (no redundant work), runs
flash-style attention, and computes a partial output projection over its 128
cat dims. The host sums the 4 partial outputs per batch and adds bo.

exp() is split between the scalar engine (exact LUT) and the vector engine
(calibrated fp16 Schraudolph int-trick) to balance the two bottleneck
engines; OFFLOAD/16 of the score tiles take the vector path.
"""
import os
import sys

for _p in ("/opt/trn_rl_repo",):
    if _p not in sys.path:
        sys.path.insert(0, _p)

import numpy as np
from contextlib import ExitStack

import concourse.bass as bass
import concourse.bacc as bacc
import concourse.tile as tile
from concourse import mybir
from concourse.bass_utils import run_bass_kernel_spmd

F16 = mybir.dt.float16
F32 = mybir.dt.float32
I16 = mybir.dt.int16

D = 512          # d_model
DK = 64          # head dim
S = 4096         # sequence length
NCORES = 8
NSW = 8          # sweeps of 512 queries
NCH = 32         # 128-key chunks
NGRP = 8         # kv 512-key projection groups
NU = NSW * 32    # units: sweep x (16 granules x 2 heads)

# exp offload: units with u % 16 < OFFLOAD take the DVE fast-exp path
OFFLOAD = 5
LOG2E = 1.4426950408889634
EXP_A = float(0.125 * LOG2E * 1024.0)
EXP_B = float(15.0 * 1024.0 - 60.0)

LAST_RESULTS = None


def _build_kernel():
    nc = bacc.Bacc("TRN2", target_bir_lowering=False, debug=False,
                   num_devices=NCORES)

    qT = nc.dram_tensor("qT", [D, S], F16, kind="ExternalInput").ap()
    kT = nc.dram_tensor("kT", [D, S], F16, kind="ExternalInput").ap()
    vT = nc.dram_tensor("vT", [D, S], F16, kind="ExternalInput").ap()
    wq = nc.dram_tensor("wq", [D, 128], F16, kind="ExternalInput").ap()
    wk = nc.dram_tensor("wk", [D, 128], F16, kind="ExternalInput").ap()
    wv = nc.dram_tensor("wv", [D, 128], F16, kind="ExternalInput").ap()
    wo = nc.dram_tensor("wo", [128, D], F16, kind="ExternalInput").ap()
    bq = nc.dram_tensor("bq", [128], F32, kind="ExternalInput").ap()
    bk = nc.dram_tensor("bk", [128], F32, kind="ExternalInput").ap()
    bv = nc.dram_tensor("bv", [128], F32, kind="ExternalInput").ap()
    out = nc.dram_tensor("out", [S, D], F16, kind="ExternalOutput").ap()

    with tile.TileContext(nc) as tc:
        _emit(tc, qT, kT, vT, wq, wk, wv, wo, bq, bk, bv, out)

    nc.compile()
    return nc


def _emit(tc, qT, kT, vT, wq, wk, wv, wo, bq, bk, bv, out):
    nc = tc.nc
    Exp = mybir.ActivationFunctionType.Exp

    with ExitStack() as ctx:
        const = ctx.enter_context(tc.tile_pool(name="const", bufs=1))
        kvin = ctx.enter_context(tc.tile_pool(name="kvin", bufs=3))
        qinp = ctx.enter_context(tc.tile_pool(name="qinp", bufs=2))
        qhp = ctx.enter_context(tc.tile_pool(name="qhp", bufs=2))
        catp = ctx.enter_context(tc.tile_pool(name="catp", bufs=2))
        ptp = ctx.enter_context(tc.tile_pool(name="ptp", bufs=5))
        normp = ctx.enter_context(tc.tile_pool(name="normp", bufs=2))
        obp = ctx.enter_context(tc.tile_pool(name="obp", bufs=2))
        # PSUM: 3 x [128,1024] score tiles (6 banks) + 2 pv accumulators.
        # Projection tiles borrow "st" slots.
        stp = ctx.enter_context(tc.tile_pool(name="stp", bufs=3, space="PSUM"))
        pvpp = ctx.enter_context(tc.tile_pool(name="pvpp", bufs=2, space="PSUM"))
        pjp = stp

        # ---- persistent SBUF tensors -------------------------------------
        wq_sb = const.tile([128, 4 * 128], F16)   # [dm%128, m*128 + d2h]
        wk_sb = const.tile([128, 4 * 128], F16)
        wv_sb = const.tile([128, 4 * 128], F16)
        wo_sb = const.tile([128, D], F16)         # [cat dim, e]
        khT = const.tile([128, S], F16)           # [d2h, keys]
        vh = const.tile([128, NCH * 130], F16)    # per chunk: j*65+dd, col 64=1
        bqk = const.tile([128, 2], F32)           # col0 bq, col1 bk
        bv_sb = const.tile([1, 128], F32)
        bv_rep = const.tile([128, 128], F32)
        ones32 = const.tile([1, 128], F32)

        # ---- startup loads ----------------------------------------------
        nc.sync.dma_start(wk_sb[:].rearrange("p (m d) -> p m d", d=128),
                          wk.rearrange("(m p) d -> p m d", p=128))
        nc.sync.dma_start(wv_sb[:].rearrange("p (m d) -> p m d", d=128),
                          wv.rearrange("(m p) d -> p m d", p=128))
        nc.gpsimd.dma_start(wq_sb[:].rearrange("p (m d) -> p m d", d=128),
                            wq.rearrange("(m p) d -> p m d", p=128))
        nc.gpsimd.dma_start(wo_sb[:], wo)
        nc.sync.dma_start(bqk[:, 0:1], bq.rearrange("(a p) -> p a", p=128))
        nc.sync.dma_start(bqk[:, 1:2], bk.rearrange("(a p) -> p a", p=128))
        nc.sync.dma_start(bv_sb[0:1, :], bv.rearrange("(a d) -> a d", a=1))
        nc.vector.memset(ones32[:], 1.0)
        # ones column of every vh chunk (col 64 of each head block)
        vh_ones = vh[:, :].rearrange("p (a c) -> p a c", c=65)[:, :, 64:65]
        nc.vector.memset(vh_ones, 1.0)
        # preload the exp table set while startup DMAs run
        warm = normp.tile([1, 128], F16, tag="warm")
        nc.scalar.activation(warm[0:1, :], ones32[0:1, :], Exp)
        nc.gpsimd.partition_broadcast(bv_rep[:, :], bv_sb[0:1, :])

        # ---- DMA: 512-key kv group / 512-query group (one start each) ----
        kv_tiles = {}

        def prefetch_kv(G, eng):
            kin = kvin.tile([128, 2048], F16, tag="kin", name="kin")
            vin = kvin.tile([128, 2048], F16, tag="vin", name="vin")
            kv_tiles[G] = (kin, vin)
            src_k = kT.rearrange("(m p) k -> p m k", p=128)[:, :, G * 512:(G + 1) * 512]
            src_v = vT.rearrange("(m p) k -> p m k", p=128)[:, :, G * 512:(G + 1) * 512]
            eng.dma_start(kin[:].rearrange("p (m k) -> p m k", k=512), src_k)
            eng.dma_start(vin[:].rearrange("p (m k) -> p m k", k=512), src_v)

        qin_tiles = {}

        def prefetch_q(s, eng):
            qin = qinp.tile([128, 2048], F16, tag="qin", name="qin")
            qin_tiles[s] = qin
            src = qT.rearrange("(m p) k -> p m k", p=128)[:, :, s * 512:(s + 1) * 512]
            eng.dma_start(qin[:].rearrange("p (m k) -> p m k", k=512), src)

        # ---- projections --------------------------------------------------
        qh_tiles = {}

        def emit_qproj(s):
            qin = qin_tiles.pop(s)
            qh = qhp.tile([128, 512], F16, tag="qh", name="qh")
            qh_tiles[s] = qh
            pj = pjp.tile([128, 512], F32, tag="st", name="pj")
            for m in range(4):
                nc.tensor.matmul(
                    pj[:],
                    wq_sb[:, m * 128:(m + 1) * 128],
                    qin[:, m * 512:(m + 1) * 512],
                    start=(m == 0), stop=(m == 3))
            nc.vector.tensor_scalar_add(qh[:], pj[:], bqk[:, 0:1])

        def emit_kproj(G):
            kin, _ = kv_tiles[G]
            pj = pjp.tile([128, 512], F32, tag="st", name="pj")
            for m in range(4):
                nc.tensor.matmul(
                    pj[:],
                    wk_sb[:, m * 128:(m + 1) * 128],
                    kin[:, m * 512:(m + 1) * 512],
                    start=(m == 0), stop=(m == 3))
            nc.vector.tensor_scalar_add(
                khT[:, G * 512:(G + 1) * 512], pj[:], bqk[:, 1:2])

        def emit_vproj(G, ci):
            _, vin = kv_tiles[G]
            cc = 4 * G + ci
            pj = pjp.tile([128, 128], F32, tag="st", name="pj")
            for m in range(4):
                nc.tensor.matmul(
                    pj[:],
                    vin[:, m * 512 + ci * 128: m * 512 + ci * 128 + 128],
                    wv_sb[:, m * 128:(m + 1) * 128],
                    start=(m == 0), stop=(m == 3))
            dst = vh[:, cc * 130:(cc + 1) * 130]
            dst = dst.rearrange("p (h c) -> p h c", c=65)[:, :, 0:64]
            nc.vector.tensor_add(
                dst,
                pj[:].rearrange("p (h c) -> p h c", c=64),
                bv_rep[:].rearrange("p (h c) -> p h c", c=64))

        # ---- output projection (partial: 128 cat dims) --------------------
        cat_tiles = {}
        ob_tiles = {}

        def emit_oproj(s, i):
            cat = cat_tiles[s]
            if i == 0:
                ob_tiles[s] = obp.tile([128, 2048], F16, tag="ob", name="ob")
            ob = ob_tiles[s]
            pj = pjp.tile([128, 512], F32, tag="st", name="pj")
            nc.tensor.matmul(pj[:], cat[:, i * 128:(i + 1) * 128], wo_sb[:])
            nc.vector.tensor_copy(ob[:, i * 512:(i + 1) * 512], pj[:])
            if i == 3:
                cat_tiles.pop(s)
                ob = ob_tiles.pop(s)
                dst = out[s * 512:(s + 1) * 512, :].rearrange(
                    "(i p) e -> p i e", p=128)
                nc.sync.dma_start(
                    dst, ob[:].rearrange("p (i e) -> p i e", e=512))

        # ---- attention pipeline -------------------------------------------
        # unit u: sweep s = u//32, g = (u%32)//2, head j = u%2
        # stages: SC at u, exp at u-2, PV at u-4
        pvp_tiles = {}
        st_tiles = {}
        pt_tiles = {}

        def unit(u):
            s, r = divmod(u, 32)
            return s, r // 2, r % 2

        def emit_sc(u):
            s, g, j = unit(u)
            qh = qh_tiles[s]
            stt = stp.tile([128, 1024], F32, tag="st", name="stt")
            st_tiles[u] = stt
            lo = 64 * j
            for ci in range(2):
                cc = 2 * g + ci
                nc.tensor.matmul(
                    stt[:, ci * 512:(ci + 1) * 512],
                    khT[lo:lo + 64, cc * 128:(cc + 1) * 128],
                    qh[lo:lo + 64, :],
                    tile_position=(lo, 0))

        def emit_act(u):
            ptt = ptp.tile([128, 1024], F16, tag="pt", name="pt")
            pt_tiles[u] = ptt
            stt = st_tiles.pop(u)
            if u % 16 < OFFLOAD:
                nc.vector.tensor_scalar(
                    ptt[:].bitcast(I16), stt[:], EXP_A, EXP_B,
                    mybir.AluOpType.mult, mybir.AluOpType.add)
            else:
                nc.scalar.activation(ptt[:], stt[:], Exp, scale=0.125)

        def emit_pv(u):
            s, g, j = unit(u)
            if g == 0:
                pvp_tiles[j] = pvpp.tile([128, 512], F32, tag="pvp", name="pvp")
            pvp = pvp_tiles[j]
            ptt = pt_tiles.pop(u)
            for ci in range(2):
                cc = 2 * g + ci
                nc.tensor.matmul(
                    pvp[0:65, :],
                    vh[:, cc * 130 + 65 * j: cc * 130 + 65 * j + 65],
                    ptt[:, ci * 512:(ci + 1) * 512],
                    start=(g == 0 and ci == 0), stop=(g == 15 and ci == 1))
            if g == 15:
                if j == 0:
                    cat_tiles[s] = catp.tile([128, 512], F16, tag="cat",
                                             name="cat")
                cat = cat_tiles[s]
                sums = normp.tile([1, 512], F32, tag="sums")
                nc.vector.tensor_copy(sums[0:1, :], pvp[64:65, :])
                rec = normp.tile([1, 512], F32, tag="rec")
                nc.vector.reciprocal_approx_fast(rec[0:1, :], sums[0:1, :])
                rep = normp.tile([64, 512], F32, tag="rep")
                nc.gpsimd.partition_broadcast(rep[:, :], rec[0:1, :])
                lo = 64 * j
                nc.vector.tensor_mul(cat[lo:lo + 64, :], pvp[0:64, :], rep[:])

        # ---- schedule ------------------------------------------------------
        prefetch_kv(0, nc.sync)
        prefetch_kv(1, nc.gpsimd)
        prefetch_kv(2, nc.sync)
        prefetch_q(0, nc.gpsimd)
        prefetch_q(1, nc.sync)

        for u in range(NU + 4):
            if u < NU:
                s, r = u // 32, u % 32
                if u == 0:
                    emit_qproj(0)
                    emit_kproj(0)
                    for ci in range(4):
                        emit_vproj(0, ci)
                # kv projection: group G over units 4(G-1)+1 .. 4(G-1)+4
                if 1 <= u <= 4 * (NGRP - 1):
                    G, step = (u - 1) // 4 + 1, (u - 1) % 4
                    if step == 0:
                        emit_kproj(G)
                    emit_vproj(G, step)
                    if step == 3 and G + 2 < NGRP:
                        prefetch_kv(G + 2, (nc.sync, nc.gpsimd)[G % 2])
                if r == 2 and s + 2 < NSW:
                    prefetch_q(s + 2, (nc.gpsimd, nc.sync)[s % 2])
                if r == 16 and s + 1 < NSW:
                    emit_qproj(s + 1)
                if s >= 1 and r in (6, 12, 18, 24):
                    emit_oproj(s - 1, r // 6 - 1)
            if u >= 4:
                emit_pv(u - 4)
            if u < NU:
                emit_sc(u)
            if u >= 2 and u - 2 < NU:
                emit_act(u - 2)

        for i in range(4):
            emit_oproj(NSW - 1, i)


_NC_CACHE = None


def _get_nc():
    global _NC_CACHE
    if _NC_CACHE is None:
        _NC_CACHE = _build_kernel()
    return _NC_CACHE


def kernel(q, k, v, Wq, bq, Wk, bk, Wv, bv, Wo, bo, trace=False):
    global LAST_RESULTS
    q = np.asarray(q, np.float32)
    k = np.asarray(k, np.float32)
    v = np.asarray(v, np.float32)

    qT16 = [np.ascontiguousarray(q[b].T).astype(np.float16) for b in range(2)]
    kT16 = [np.ascontiguousarray(k[b].T).astype(np.float16) for b in range(2)]
    vT16 = [np.ascontiguousarray(v[b].T).astype(np.float16) for b in range(2)]
    WqT = np.asarray(Wq, np.float32).T
    WkT = np.asarray(Wk, np.float32).T
    WvT = np.asarray(Wv, np.float32).T
    WoT = np.asarray(Wo, np.float32).T
    bq32 = np.asarray(bq, np.float32)
    bk32 = np.asarray(bk, np.float32)
    bv32 = np.asarray(bv, np.float32)

    in_maps = []
    for core in range(NCORES):
        b, hp = divmod(core, 4)
        sl = slice(128 * hp, 128 * (hp + 1))
        in_maps.append({
            "qT": qT16[b], "kT": kT16[b], "vT": vT16[b],
            "wq": np.ascontiguousarray(WqT[:, sl]).astype(np.float16),
            "wk": np.ascontiguousarray(WkT[:, sl]).astype(np.float16),
            "wv": np.ascontiguousarray(WvT[:, sl]).astype(np.float16),
            "wo": np.ascontiguousarray(WoT[sl, :]).astype(np.float16),
            "bq": np.ascontiguousarray(bq32[sl]),
            "bk": np.ascontiguousarray(bk32[sl]),
            "bv": np.ascontiguousarray(bv32[sl]),
        })

    nc = _get_nc()
    res = run_bass_kernel_spmd(nc, in_maps, core_ids=list(range(NCORES)),
                               trace=trace)
    LAST_RESULTS = res

    full = np.zeros((2, S, D), np.float32)
    for core in range(NCORES):
        b, hp = divmod(core, 4)
        full[b] += res.results[core]["out"].astype(np.float32)
    full += np.asarray(bo, np.float32)
    return full


# revision 10
# speedup vs baseline: 1.0644x; 1.0320x over previous
"""Multi-head attention (B=2, S=4096, D=512, H=8) on 8 TRN2 NeuronCores.

Sharding: core = (batch, head-pair). Each core projects q/k/v onto its two
heads' 128 dims over the full 4096-token sequence (no redundant work), runs
flash-style attention, and computes a partial output projection over its 128
cat dims. The host sums the 4 partial outputs per batch and adds bo.

exp() is split between the scalar engine (exact LUT) and the vector engine
(calibrated fp16 Schraudolph int-trick) to balance the two bottleneck
engines; OFFLOAD/16 of the score tiles take the vector path.
"""
import os
import sys

for _p in ("/opt/trn_rl_repo",):
    if _p not in sys.path:
        sys.path.insert(0, _p)

import numpy as np
from contextlib import ExitStack

import concourse.bass as bass
import concourse.bacc as bacc
import concourse.tile as tile
from concourse import mybir
from concourse.bass_utils import run_bass_kernel_spmd

F16 = mybir.dt.float16
F32 = mybir.dt.float32
I16 = mybir.dt.int16

D = 512          # d_model
DK = 64          # head dim
S = 4096         # sequence length
NCORES = 8
NSW = 8          # sweeps of 512 queries
NCH = 32         # 128-key chunks
NGRP = 8         # kv 512-key projection groups
NU = NSW * 32    # units: sweep x (16 granules x 2 heads)

# exp offload: units with u % 16 < OFFLOAD take the DVE fast-exp path
OFFLOAD = 7
LOG2E = 1.4426950408889634
EXP_A = float(0.125 * LOG2E * 1024.0)
EXP_B = float(15.0 * 1024.0 - 60.0)

LAST_RESULTS = None


def _build_kernel():
    nc = bacc.Bacc("TRN2", target_bir_lowering=False, debug=False,
                   num_devices=NCORES)

    qT = nc.dram_tensor("qT", [D, S], F16, kind="ExternalInput").ap()
    kT = nc.dram_tensor("kT", [D, S], F16, kind="ExternalInput").ap()
    vT = nc.dram_tensor("vT", [D, S], F16, kind="ExternalInput").ap()
    wq = nc.dram_tensor("wq", [D, 128], F16, kind="ExternalInput").ap()
    wk = nc.dram_tensor("wk", [D, 128], F16, kind="ExternalInput").ap()
    wv = nc.dram_tensor("wv", [D, 128], F16, kind="ExternalInput").ap()
    wo = nc.dram_tensor("wo", [128, D], F16, kind="ExternalInput").ap()
    bq = nc.dram_tensor("bq", [128], F32, kind="ExternalInput").ap()
    bk = nc.dram_tensor("bk", [128], F32, kind="ExternalInput").ap()
    bv = nc.dram_tensor("bv", [128], F32, kind="ExternalInput").ap()
    out = nc.dram_tensor("out", [S, D], F16, kind="ExternalOutput").ap()

    with tile.TileContext(nc) as tc:
        _emit(tc, qT, kT, vT, wq, wk, wv, wo, bq, bk, bv, out)

    nc.compile()
    return nc


def _emit(tc, qT, kT, vT, wq, wk, wv, wo, bq, bk, bv, out):
    nc = tc.nc
    Exp = mybir.ActivationFunctionType.Exp

    with ExitStack() as ctx:
        const = ctx.enter_context(tc.tile_pool(name="const", bufs=1))
        kvin = ctx.enter_context(tc.tile_pool(name="kvin", bufs=3))
        qinp = ctx.enter_context(tc.tile_pool(name="qinp", bufs=2))
        qhp = ctx.enter_context(tc.tile_pool(name="qhp", bufs=2))
        catp = ctx.enter_context(tc.tile_pool(name="catp", bufs=2))
        ptp = ctx.enter_context(tc.tile_pool(name="ptp", bufs=5))
        normp = ctx.enter_context(tc.tile_pool(name="normp", bufs=2))
        obp = ctx.enter_context(tc.tile_pool(name="obp", bufs=2))
        # PSUM: 3 x [128,1024] score tiles (6 banks) + 2 pv accumulators.
        # Projection tiles borrow "st" slots.
        stp = ctx.enter_context(tc.tile_pool(name="stp", bufs=3, space="PSUM"))
        pvpp = ctx.enter_context(tc.tile_pool(name="pvpp", bufs=2, space="PSUM"))
        pjp = stp

        # ---- persistent SBUF tensors -------------------------------------
        wq_sb = const.tile([128, 4 * 128], F16)   # [dm%128, m*128 + d2h]
        wk_sb = const.tile([128, 4 * 128], F16)
        wv_sb = const.tile([128, 4 * 128], F16)
        wo_sb = const.tile([128, D], F16)         # [cat dim, e]
        khT = const.tile([128, S], F16)           # [d2h, keys]
        vh = const.tile([128, NCH * 130], F16)    # per chunk: j*65+dd, col 64=1
        bqk = const.tile([128, 2], F32)           # col0 bq, col1 bk
        bv_sb = const.tile([1, 128], F32)
        bv_rep = const.tile([128, 128], F32)
        ones32 = const.tile([1, 128], F32)

        # ---- startup loads ----------------------------------------------
        nc.sync.dma_start(wk_sb[:].rearrange("p (m d) -> p m d", d=128),
                          wk.rearrange("(m p) d -> p m d", p=128))
        nc.sync.dma_start(wv_sb[:].rearrange("p (m d) -> p m d", d=128),
                          wv.rearrange("(m p) d -> p m d", p=128))
        nc.gpsimd.dma_start(wq_sb[:].rearrange("p (m d) -> p m d", d=128),
                            wq.rearrange("(m p) d -> p m d", p=128))
        nc.gpsimd.dma_start(wo_sb[:], wo)
        nc.sync.dma_start(bqk[:, 0:1], bq.rearrange("(a p) -> p a", p=128))
        nc.sync.dma_start(bqk[:, 1:2], bk.rearrange("(a p) -> p a", p=128))
        nc.sync.dma_start(bv_sb[0:1, :], bv.rearrange("(a d) -> a d", a=1))
        nc.vector.memset(ones32[:], 1.0)
        # ones column of every vh chunk (col 64 of each head block)
        vh_ones = vh[:, :].rearrange("p (a c) -> p a c", c=65)[:, :, 64:65]
        nc.vector.memset(vh_ones, 1.0)
        # preload the exp table set while startup DMAs run
        warm = normp.tile([1, 128], F16, tag="warm")
        nc.scalar.activation(warm[0:1, :], ones32[0:1, :], Exp)
        nc.gpsimd.partition_broadcast(bv_rep[:, :], bv_sb[0:1, :])

        # ---- DMA: 512-key kv group / 512-query group (one start each) ----
        kv_tiles = {}

        def prefetch_kv(G, eng, split=None):
            kin = kvin.tile([128, 2048], F16, tag="kin", name="kin")
            vin = kvin.tile([128, 2048], F16, tag="vin", name="vin")
            kv_tiles[G] = (kin, vin)
            src_k = kT.rearrange("(m p) k -> p m k", p=128)[:, :, G * 512:(G + 1) * 512]
            src_v = vT.rearrange("(m p) k -> p m k", p=128)[:, :, G * 512:(G + 1) * 512]
            dst_k = kin[:].rearrange("p (m k) -> p m k", k=512)
            dst_v = vin[:].rearrange("p (m k) -> p m k", k=512)
            if split is not None:
                # halve the critical first group across two queues
                eng.dma_start(dst_k[:, 0:2], src_k[:, 0:2])
                split.dma_start(dst_k[:, 2:4], src_k[:, 2:4])
                eng.dma_start(dst_v[:, 0:2], src_v[:, 0:2])
                split.dma_start(dst_v[:, 2:4], src_v[:, 2:4])
            else:
                eng.dma_start(dst_k, src_k)
                eng.dma_start(dst_v, src_v)

        qin_tiles = {}

        def prefetch_q(s, eng):
            qin = qinp.tile([128, 2048], F16, tag="qin", name="qin")
            qin_tiles[s] = qin
            src = qT.rearrange("(m p) k -> p m k", p=128)[:, :, s * 512:(s + 1) * 512]
            eng.dma_start(qin[:].rearrange("p (m k) -> p m k", k=512), src)

        # ---- projections --------------------------------------------------
        qh_tiles = {}

        def emit_qproj(s):
            qin = qin_tiles.pop(s)
            qh = qhp.tile([128, 512], F16, tag="qh", name="qh")
            qh_tiles[s] = qh
            pj = pjp.tile([128, 512], F32, tag="st", name="pj")
            for m in range(4):
                nc.tensor.matmul(
                    pj[:],
                    wq_sb[:, m * 128:(m + 1) * 128],
                    qin[:, m * 512:(m + 1) * 512],
                    start=(m == 0), stop=(m == 3))
            nc.scalar.add(qh[:], pj[:], bqk[:, 0:1])

        def emit_kproj(G):
            kin, _ = kv_tiles[G]
            pj = pjp.tile([128, 512], F32, tag="st", name="pj")
            for m in range(4):
                nc.tensor.matmul(
                    pj[:],
                    wk_sb[:, m * 128:(m + 1) * 128],
                    kin[:, m * 512:(m + 1) * 512],
                    start=(m == 0), stop=(m == 3))
            nc.scalar.add(khT[:, G * 512:(G + 1) * 512], pj[:], bqk[:, 1:2])

        def emit_vproj(G, ci):
            _, vin = kv_tiles[G]
            cc = 4 * G + ci
            pj = pjp.tile([128, 128], F32, tag="st", name="pj")
            for m in range(4):
                nc.tensor.matmul(
                    pj[:],
                    vin[:, m * 512 + ci * 128: m * 512 + ci * 128 + 128],
                    wv_sb[:, m * 128:(m + 1) * 128],
                    start=(m == 0), stop=(m == 3))
            dst = vh[:, cc * 130:(cc + 1) * 130]
            dst = dst.rearrange("p (h c) -> p h c", c=65)[:, :, 0:64]
            nc.vector.tensor_add(
                dst,
                pj[:].rearrange("p (h c) -> p h c", c=64),
                bv_rep[:].rearrange("p (h c) -> p h c", c=64))

        # ---- output projection (partial: 128 cat dims) --------------------
        cat_tiles = {}
        ob_tiles = {}

        def emit_oproj(s, i):
            cat = cat_tiles[s]
            if i == 0:
                ob_tiles[s] = obp.tile([128, 2048], F16, tag="ob", name="ob")
            ob = ob_tiles[s]
            pj = pjp.tile([128, 512], F32, tag="st", name="pj")
            nc.tensor.matmul(pj[:], cat[:, i * 128:(i + 1) * 128], wo_sb[:])
            nc.scalar.copy(ob[:, i * 512:(i + 1) * 512], pj[:])
            if i == 3:
                cat_tiles.pop(s)
                ob = ob_tiles.pop(s)
                dst = out[s * 512:(s + 1) * 512, :].rearrange(
                    "(i p) e -> p i e", p=128)
                nc.sync.dma_start(
                    dst, ob[:].rearrange("p (i e) -> p i e", e=512))

        # ---- attention pipeline -------------------------------------------
        # unit u: sweep s = u//32, g = (u%32)//2, head j = u%2
        # stages: SC at u, exp at u-2, PV at u-4
        pvp_tiles = {}
        st_tiles = {}
        pt_tiles = {}

        def unit(u):
            s, r = divmod(u, 32)
            return s, r // 2, r % 2

        def emit_sc(u):
            s, g, j = unit(u)
            qh = qh_tiles[s]
            stt = stp.tile([128, 1024], F32, tag="st", name="stt")
            st_tiles[u] = stt
            lo = 64 * j
            for ci in range(2):
                cc = 2 * g + ci
                nc.tensor.matmul(
                    stt[:, ci * 512:(ci + 1) * 512],
                    khT[lo:lo + 64, cc * 128:(cc + 1) * 128],
                    qh[lo:lo + 64, :],
                    tile_position=(lo, 0))

        def emit_act(u):
            ptt = ptp.tile([128, 1024], F16, tag="pt", name="pt")
            pt_tiles[u] = ptt
            stt = st_tiles.pop(u)
            if u % 16 < OFFLOAD:
                nc.vector.tensor_scalar(
                    ptt[:].bitcast(I16), stt[:], EXP_A, EXP_B,
                    mybir.AluOpType.mult, mybir.AluOpType.add)
            else:
                nc.scalar.activation(ptt[:], stt[:], Exp, scale=0.125)

        def emit_pv(u):
            s, g, j = unit(u)
            if g == 0:
                pvp_tiles[j] = pvpp.tile([128, 512], F32, tag="pvp", name="pvp")
            pvp = pvp_tiles[j]
            ptt = pt_tiles.pop(u)
            for ci in range(2):
                cc = 2 * g + ci
                nc.tensor.matmul(
                    pvp[0:65, :],
                    vh[:, cc * 130 + 65 * j: cc * 130 + 65 * j + 65],
                    ptt[:, ci * 512:(ci + 1) * 512],
                    start=(g == 0 and ci == 0), stop=(g == 15 and ci == 1))
            if g == 15:
                if j == 0:
                    cat_tiles[s] = catp.tile([128, 512], F16, tag="cat",
                                             name="cat")
                cat = cat_tiles[s]
                sums = normp.tile([1, 512], F32, tag="sums")
                nc.vector.tensor_copy(sums[0:1, :], pvp[64:65, :])
                rec = normp.tile([1, 512], F32, tag="rec")
                nc.vector.reciprocal_approx_fast(rec[0:1, :], sums[0:1, :])
                rep = normp.tile([64, 512], F32, tag="rep")
                nc.gpsimd.partition_broadcast(rep[:, :], rec[0:1, :])
                lo = 64 * j
                nc.vector.tensor_mul(cat[lo:lo + 64, :], pvp[0:64, :], rep[:])

        # ---- schedule ------------------------------------------------------
        prefetch_kv(0, nc.sync, split=nc.gpsimd)
        prefetch_kv(1, nc.gpsimd)
        prefetch_kv(2, nc.sync)
        prefetch_q(0, nc.gpsimd)
        prefetch_q(1, nc.sync)

        def hooks(u):
            s, r = u // 32, u % 32
            if u == 0:
                emit_qproj(0)
                emit_kproj(0)
                for ci in range(4):
                    emit_vproj(0, ci)
            # kv projection: group G over units 4(G-1)+1 .. 4(G-1)+4
            if 1 <= u <= 4 * (NGRP - 1):
                G, step = (u - 1) // 4 + 1, (u - 1) % 4
                if step == 0:
                    emit_kproj(G)
                emit_vproj(G, step)
                if step == 3 and G + 2 < NGRP:
                    prefetch_kv(G + 2, (nc.sync, nc.gpsimd)[G % 2])
            if r == 2 and s + 2 < NSW:
                prefetch_q(s + 2, (nc.gpsimd, nc.sync)[s % 2])
            if r == 16 and s + 1 < NSW:
                emit_qproj(s + 1)
            if s >= 1 and r in (6, 12, 18, 24):
                emit_oproj(s - 1, r // 6 - 1)

        # pair-batched pipeline: PV(p-4,p-3) | ACT(p-2,p-1) | SC(p,p+1).
        # SC pairs are adjacent in the tensor queue so the two heads' row
        # groups run concurrently; PV runs of 4 expose only one LDWEIGHTS.
        for p in range(0, NU + 4, 2):
            u0, u1 = p, p + 1
            if u0 < NU:
                hooks(u0)
                hooks(u1)
            if u0 >= 4:
                emit_pv(u0 - 4)
                emit_pv(u1 - 4)
            if u0 >= 2 and u0 - 2 < NU:
                emit_act(u0 - 2)
                emit_act(u1 - 2)
            if u0 < NU:
                emit_sc(u0)
                emit_sc(u1)

        for i in range(4):
            emit_oproj(NSW - 1, i)


_NC_CACHE = None


def _get_nc():
    global _NC_CACHE
    if _NC_CACHE is None:
        _NC_CACHE = _build_kernel()
    return _NC_CACHE


def kernel(q, k, v, Wq, bq, Wk, bk, Wv, bv, Wo, bo, trace=False):
    global LAST_RESULTS
    q = np.asarray(q, np.float32)
    k = np.asarray(k, np.float32)
    v = np.asarray(v, np.float32)

    qT16 = [np.ascontiguousarray(q[b].T).astype(np.float16) for b in range(2)]
    kT16 = [np.ascontiguousarray(k[b].T).astype(np.float16) for b in range(2)]
    vT16 = [np.ascontiguousarray(v[b].T).astype(np.float16) for b in range(2)]
    WqT = np.asarray(Wq, np.float32).T
    WkT = np.asarray(Wk, np.float32).T
    WvT = np.asarray(Wv, np.float32).T
    WoT = np.asarray(Wo, np.float32).T
    bq32 = np.asarray(bq, np.float32)
    bk32 = np.asarray(bk, np.float32)
    bv32 = np.asarray(bv, np.float32)

    in_maps = []
    for core in range(NCORES):
        b, hp = divmod(core, 4)
        sl = slice(128 * hp, 128 * (hp + 1))
        in_maps.append({
            "qT": qT16[b], "kT": kT16[b], "vT": vT16[b],
            "wq": np.ascontiguousarray(WqT[:, sl]).astype(np.float16),
            "wk": np.ascontiguousarray(WkT[:, sl]).astype(np.float16),
            "wv": np.ascontiguousarray(WvT[:, sl]).astype(np.float16),
            "wo": np.ascontiguousarray(WoT[sl, :]).astype(np.float16),
            "bq": np.ascontiguousarray(bq32[sl]),
            "bk": np.ascontiguousarray(bk32[sl]),
            "bv": np.ascontiguousarray(bv32[sl]),
        })

    nc = _get_nc()
    res = run_bass_kernel_spmd(nc, in_maps, core_ids=list(range(NCORES)),
                               trace=trace)
    LAST_RESULTS = res

    full = np.zeros((2, S, D), np.float32)
    for core in range(NCORES):
        b, hp = divmod(core, 4)
        full[b] += res.results[core]["out"].astype(np.float32)
    full += np.asarray(bo, np.float32)
    return full
